# revision 41
# baseline (speedup 1.0000x reference)
"""GAE (Generalized Advantage Estimation) Bass kernel for 8 Trainium2 cores.

Problem: rewards (2048, 8192) f32, values (2048, 8192) f32,
next_values (2048,) f32.
  next_v[:, t] = values[:, t+1] (t < S-1), next_values (t = S-1)
  deltas = rewards + GAMMA * next_v - values  # (B, S)
  A_t = deltas_t + (GAMMA*LAM) * A_{t+1}   (A_S = 0, backward recurrence)
  advantages = A, returns = A + values

Sharding: pure data parallel over the batch dim — 2048 rows / 8 cores =
256 rows per core; the seq recurrence is row-local so there is no
cross-core communication.

Algorithm: the scan coefficient c = GAMMA*LAM = 0.9405 decays fast
(c^129 ~ 3.7e-4), so the backward recurrence is a finite-horizon
convolution: with the change of variable e'_t = r_t + k*v_t
(k = (1-LAM)/LAM) and a phantom tail element e'_S = nv/LAM,
  C_t   = sum_{u>=t} c^(u-t) e'_u,   adv = C - v/LAM,  ret = C - k*v.
Lay the seq axis over partitions in 128-step blocks (host packs
[rows, S] -> [128, 64*rows]); then C for one block is two matmuls with
constant 128x128 Toeplitz weights:
  C_i = T0 @ e'_i + T1 @ e'_{i+1},  T0[j,i] = c^(j-i) (j>=i), T1 = c^128*c^(j-i)
which runs on the otherwise-idle PE and removes the serial
tensor_tensor_scan (the baseline's DVE bottleneck: 36us of scans + 21us
of TTs on one engine) entirely.  The -k*v term of ret folds into the
weights (W0m = T0 - I applied to the host-sent m = k*v), so PSUM
accumulates ret directly; adv = ret - (1/(1-LAM)-1)*m is one DVE
scalar_tensor_tensor from PSUM and ret drains through the ScalarE.
Everything moves as fp16 (better mantissa than bf16 at the same byte
cost; weights below 1e-4 are clamped to zero to dodge fp16 subnormals),
and the truncation error is ~2e-4 against a 2e-2 gate (measured 5e-4
total vs the baseline's 5.3e-3).

Per supergroup of 4 PSUM banks (2048 cols = 8 seq-blocks x 256 rows):
16 matmuls (4 weight passes) into two 2-bank PSUM tiles (so the two
drain engines track as disjoint tensors and run concurrently), ScalarE
copies ret while the DVE STTs adv on the opposite half, then each half
stores to the opposite HWDGE ring.  Ring discipline (HWDGE rings are
FIFO and wedge if directions interleave): rp + half the stores ride
sync, mp + the other half ride scalar, so each ring carries loads
back-to-back, flips direction exactly once, and streams stores whose
drains are always already complete — both rings measure ~410-420 GB/s
combined end-to-end, which is the HBM wall for the 17MB moved.  Load
chunks are finer (1024 cols) for the first 4096 cols so the PE's first
supergroups aren't starved during the DMA ramp (measured: 2048-only
costs +6us, SWDGE/gpsimd queues are far slower — don't).  A few dummy
matmuls + one dummy activation at t~0 pre-warm the PE HAM clock gate
and the ACT table while the first loads land; GpSimd stays idle (its
TT is 4x slower and degrades concurrent DVE SBUF ops).
"""

import sys

if "/opt/trn_rl_repo" not in sys.path:
    sys.path.insert(0, "/opt/trn_rl_repo")

import numpy as np

GAMMA = 0.99
LAM = 0.95
C_COEF = GAMMA * LAM
K_COEF = (1.0 - LAM) / LAM  # e' = r + k*v ; ret = C - k*v
R_COEF = 1.0 / (1.0 - LAM) - 1.0  # adv = ret - R_COEF * (k*v)

B, S = 2048, 8192
N_CORES = 8
ROWS = B // N_CORES  # 256 rows per core
P = 128  # SBUF partitions = seq positions per block
NB = S // P  # 64 seq blocks per core
NCOL = NB * ROWS  # 16384 packed columns
PHW = ROWS  # phantom block width (256)
SG = 2048  # supergroup: 4 PSUM banks
N_SG = NCOL // SG  # 8 supergroups
WCLAMP = 1e-4  # zero fp16-subnormal-ish weights (horizon stays ~150+)

_CACHE: dict = {}


def _weights():
    idx = np.arange(P)
    d = idx[:, None] - idx[None, :]  # j - i  (j = contraction partition)
    w0 = np.where(d >= 0, C_COEF ** np.maximum(d, 0), 0.0)  # T0 (with diag)
    w0m = np.where(d > 0, C_COEF ** np.maximum(d, 0), 0.0)  # T0 - I
    w1 = C_COEF ** (128.0 + d)
    out = []
    for w in (w0, w0m, w1):
        w[np.abs(w) < WCLAMP] = 0.0
        out.append(w.astype(np.float16))
    return out


def _build():
    import concourse.bacc as bacc
    import concourse.mybir as mybir
    from concourse.tile import TileContext

    f16 = mybir.dt.float16
    f32 = mybir.dt.float32
    add = mybir.AluOpType.add
    mult = mybir.AluOpType.mult
    Copy = mybir.ActivationFunctionType.Copy

    i8 = mybir.dt.int8
    nc = bacc.Bacc("TRN2", target_bir_lowering=False, name="gaeconv1")
    rp = nc.dram_tensor("rp", [P, NCOL + PHW], i8, kind="ExternalInput")
    mp = nc.dram_tensor("mp", [P, NCOL + PHW], f16, kind="ExternalInput")
    scl = nc.dram_tensor("scl", [P, 2], f32, kind="ExternalInput")
    w0_d = nc.dram_tensor("w0", [P, P], f16, kind="ExternalInput")
    w0m_d = nc.dram_tensor("w0m", [P, P], f16, kind="ExternalInput")
    w1_d = nc.dram_tensor("w1", [P, P], f16, kind="ExternalInput")
    # per supergroup g: cols [4096g, 4096g+2048) = ret, [+2048, +4096) = adv
    o2d = nc.dram_tensor("o2", [P, 2 * NCOL], f16, kind="ExternalOutput")

    with TileContext(nc) as tc:
        with (
            tc.tile_pool(name="cpool", bufs=1) as cpool,
            tc.tile_pool(name="psum", bufs=2, space="PSUM") as psum,
            tc.tile_pool(name="opool", bufs=5) as opool,
        ):
            w0_t = cpool.tile([P, P], f16)
            w0m_t = cpool.tile([P, P], f16)
            w1_t = cpool.tile([P, P], f16)
            rp8_t = cpool.tile([P, NCOL + PHW], i8)
            rp_t = cpool.tile([P, NCOL + PHW], f16)
            mp_t = cpool.tile([P, NCOL + PHW], f16)
            scl_t = cpool.tile([P, 2], f32)
            scratch = cpool.tile([P, P], f16)

            nc.sync.dma_start(out=scl_t[:, :], in_=scl[:, :])
            nc.sync.dma_start(out=w0_t[:, :], in_=w0_d[:, :])
            nc.sync.dma_start(out=w0m_t[:, :], in_=w0m_d[:, :])
            nc.sync.dma_start(out=w1_t[:, :], in_=w1_d[:, :])
            # int8 loads split across both HWDGE rings (rp on sync, mp on
            # scalar); finer pieces up front so the PE's first supergroups
            # aren't starved during the DMA ramp; the tail chunk carries
            # the phantom block.
            bounds = [0, 1024, 2048, 3072, 4096, 6144, 8192, 10240, 12288,
                      14336, NCOL + PHW]
            for a, b in zip(bounds, bounds[1:]):
                nc.sync.dma_start(out=rp8_t[:, a:b], in_=rp[:, a:b])
                nc.scalar.dma_start(out=mp_t[:, a:b], in_=mp[:, a:b])
            # rp dequant (int8 -> fp16, runtime scale as per-partition AP)
            # happens on the DVE *inside* the compute loop, one supergroup
            # ahead of the PE — the DVE is strict FIFO, so emitting all
            # dequants up front would block sg0's adv drain behind the
            # last chunk's dequant.  GpSimd tensor_scalar measured
            # 14ns/elem — useless for this.
            def dequant(g):
                a = g * SG
                b = a + SG + (PHW if g == N_SG - 1 else 0)
                nc.vector.tensor_scalar_mul(
                    out=rp_t[:, a:b], in0=rp8_t[:, a:b], scalar1=scl_t[:, 0:1]
                )

            dequant(0)
            dequant(1)

            # ACT table pre-load + PE HAM warm-up while the loads land
            nc.scalar.activation(out=scratch[:, 0:1], in_=w0_t[:, 0:1], func=Copy)
            warm = psum.tile([P, SG // 2], f32, tag="psA")
            for _ in range(12):
                nc.tensor.matmul(
                    warm[:, 0:P], w0_t[:, :], w0_t[:, :], start=True, stop=True
                )

            H = SG // 2
            for g in range(1, N_SG + 1):
                X = (g - 1) * SG
                if g + 1 < N_SG:
                    dequant(g + 1)
                # two separate 2-bank PSUM tiles so the ScalarE and DVE
                # drains track as disjoint tensors and run in parallel
                # (same-tile access serializes under Tile's bank tracker)
                ps_lo = psum.tile([P, H], f32, tag="psA")
                ps_hi = psum.tile([P, H], f32, tag="psB")
                o2_t = opool.tile([P, 2 * SG], f16)
                for bk in range(4):
                    xb = X + 512 * bk
                    ph = ps_lo if bk < 2 else ps_hi
                    po = ph[:, 512 * (bk % 2) : 512 * (bk % 2) + 512]
                    nc.tensor.matmul(
                        po, w0_t[:, :], rp_t[:, xb : xb + 512],
                        start=True, stop=False,
                    )
                    nc.tensor.matmul(
                        po, w0m_t[:, :], mp_t[:, xb : xb + 512],
                        start=False, stop=False,
                    )
                    nc.tensor.matmul(
                        po, w1_t[:, :], rp_t[:, xb + PHW : xb + PHW + 512],
                        start=False, stop=False,
                    )
                    nc.tensor.matmul(
                        po, w1_t[:, :], mp_t[:, xb + PHW : xb + PHW + 512],
                        start=False, stop=True,
                    )
                # parallel drains: ACT on ps_lo while DVE works ps_hi, swap
                nc.scalar.activation(
                    out=o2_t[:, 0:H], in_=ps_lo[:, :], func=Copy
                )
                nc.vector.scalar_tensor_tensor(
                    out=o2_t[:, SG + H : 2 * SG],
                    in0=mp_t[:, X + H : X + SG],
                    scalar=-R_COEF,
                    in1=ps_hi[:, :],
                    op0=mult,
                    op1=add,
                )
                nc.scalar.activation(
                    out=o2_t[:, H:SG], in_=ps_hi[:, :], func=Copy
                )
                nc.vector.scalar_tensor_tensor(
                    out=o2_t[:, SG : SG + H],
                    in0=mp_t[:, X : X + H],
                    scalar=-R_COEF,
                    in1=ps_lo[:, :],
                    op0=mult,
                    op1=add,
                )
                # every sg's two output halves go to OPPOSITE rings, so each
                # ring carries exactly half the loads + half the stores and
                # flips direction once; drains finish ahead of the ring's
                # FIFO position, keeping both rings 100% busy to the end.
                nc.scalar.dma_start(
                    out=o2d[:, 2 * X : 2 * X + SG], in_=o2_t[:, 0:SG]
                )
                nc.sync.dma_start(
                    out=o2d[:, 2 * X + SG : 2 * X + 2 * SG],
                    in_=o2_t[:, SG : 2 * SG],
                )
    nc.finalize()
    return nc


def _get_nc():
    if "nc" not in _CACHE:
        _CACHE["nc"] = _build()
    return _CACHE["nc"]


def _pack(x):
    """[ROWS, S] -> [P, NB*ROWS] with seq in partitions per 128-block."""
    return np.ascontiguousarray(
        x.reshape(ROWS, NB, P).transpose(2, 1, 0).reshape(P, NCOL)
    )


def _unpack(xp):
    """[P, NB*ROWS] -> [ROWS, S]."""
    return xp.reshape(P, NB, ROWS).transpose(2, 1, 0).reshape(ROWS, S)


def _run(rewards, values, next_values, **spmd_kwargs):
    """Shard over cores, run the Bass kernel, return BassKernelResults."""
    from concourse.bass_utils import run_bass_kernel_spmd

    nc = _get_nc()
    r32 = np.asarray(rewards, dtype=np.float32)
    mk32 = np.asarray(values, dtype=np.float32) * np.float32(K_COEF)
    nv32 = np.asarray(next_values, dtype=np.float32)
    phv = nv32 / np.float32(LAM)
    # symmetric int8 quantization; scales from the actual data so any
    # input distribution round-trips (device dequantizes via scl AP)
    s_r = np.float32(max(np.abs(r32).max(), np.abs(phv).max(), 1e-12) / 127.0)
    s_m = np.float32(max(np.abs(mk32).max(), 1e-12) / 127.0)
    scl = np.broadcast_to(
        np.array([[s_r, s_m]], dtype=np.float32), (P, 2)
    ).copy()

    def q8(x, s):
        return np.clip(np.round(x / s), -127, 127).astype(np.int8)

    w0, w0m, w1 = _weights()
    zeros_ph = np.zeros((P, PHW), dtype=np.int8)
    in_maps = []
    for c in range(N_CORES):
        sl = slice(c * ROWS, (c + 1) * ROWS)
        ph = zeros_ph.copy()
        ph[0, :] = q8(phv[sl], s_r)
        rp = np.concatenate([_pack(q8(r32[sl], s_r)), ph], 1)
        mpk = np.concatenate(
            [_pack(mk32[sl].astype(np.float16)),
             np.zeros((P, PHW), dtype=np.float16)], 1
        )
        in_maps.append(
            {
                "rp": np.ascontiguousarray(rp),
                "mp": np.ascontiguousarray(mpk),
                "scl": scl,
                "w0": w0,
                "w0m": w0m,
                "w1": w1,
            }
        )
    return run_bass_kernel_spmd(
        nc, in_maps, core_ids=list(range(N_CORES)), **spmd_kwargs
    )


def _gather(res):
    """Unshard: de-interleave per-supergroup ret/adv, unpack, upcast."""
    returns = np.empty((B, S), dtype=np.float32)
    advantages = np.empty((B, S), dtype=np.float32)
    for c in range(N_CORES):
        sl = slice(c * ROWS, (c + 1) * ROWS)
        o2 = res.results[c]["o2"].reshape(P, N_SG, 2, SG)
        retp = np.ascontiguousarray(o2[:, :, 0, :]).reshape(P, NCOL)
        advp = np.ascontiguousarray(o2[:, :, 1, :]).reshape(P, NCOL)
        returns[sl] = _unpack(retp.astype(np.float32))
        advantages[sl] = _unpack(advp.astype(np.float32))
    return advantages, returns


def kernel(rewards, values, next_values):
    res = _run(rewards, values, next_values)
    return _gather(res)


# revision 46
# speedup vs baseline: 1.2193x; 1.2193x over previous
"""GAE (Generalized Advantage Estimation) Bass kernel for 8 Trainium2 cores.

Problem: rewards (2048, 8192) f32, values (2048, 8192) f32,
next_values (2048,) f32.
  next_v[:, t] = values[:, t+1] (t < S-1), next_values (t = S-1)
  deltas = rewards + GAMMA * next_v - values  # (B, S)
  A_t = deltas_t + (GAMMA*LAM) * A_{t+1}   (A_S = 0, backward recurrence)
  advantages = A, returns = A + values

Sharding: pure data parallel over the batch dim — 2048 rows / 8 cores =
256 rows per core; the seq recurrence is row-local so there is no
cross-core communication.

Algorithm: the scan coefficient c = GAMMA*LAM = 0.9405 decays fast
(c^129 ~ 3.7e-4), so the backward recurrence is a finite-horizon
convolution: with the change of variable e'_t = r_t + k*v_t
(k = (1-LAM)/LAM) and a phantom tail element e'_S = nv/LAM,
  C_t   = sum_{u>=t} c^(u-t) e'_u,   adv = C - v/LAM,  ret = C - k*v.
Lay the seq axis over partitions in 128-step blocks (host packs
[rows, S] -> [128, 64*rows]); then C for one block is two matmuls with
constant 128x128 Toeplitz weights:
  C_i = T0 @ e'_i + T1 @ e'_{i+1},  T0[j,i] = c^(j-i) (j>=i), T1 = c^128*c^(j-i)
which runs on the otherwise-idle PE and removes the serial
tensor_tensor_scan (the baseline's DVE bottleneck: 36us of scans + 21us
of TTs on one engine) entirely.  The -k*v term of ret folds into the
weights (W0m = T0 - I applied to the host-sent m = k*v), so PSUM
accumulates ret directly; adv = ret - (1/(1-LAM)-1)*m is one DVE
scalar_tensor_tensor from PSUM and ret drains through the ScalarE.
Everything moves as fp16 (better mantissa than bf16 at the same byte
cost; weights below 1e-4 are clamped to zero to dodge fp16 subnormals),
and the truncation error is ~2e-4 against a 2e-2 gate (measured 5e-4
total vs the baseline's 5.3e-3).

Per supergroup of 4 PSUM banks (2048 cols = 8 seq-blocks x 256 rows):
16 matmuls (4 weight passes) into two 2-bank PSUM tiles (so the two
drain engines track as disjoint tensors and run concurrently), ScalarE
copies ret while the DVE STTs adv on the opposite half, then each half
stores to the opposite HWDGE ring.  Ring discipline (HWDGE rings are
FIFO and wedge if directions interleave): rp + half the stores ride
sync, mp + the other half ride scalar, so each ring carries loads
back-to-back, flips direction exactly once, and streams stores whose
drains are always already complete — both rings measure ~410-420 GB/s
combined end-to-end, which is the HBM wall for the 17MB moved.  Load
chunks are finer (1024 cols) for the first 4096 cols so the PE's first
supergroups aren't starved during the DMA ramp (measured: 2048-only
costs +6us, SWDGE/gpsimd queues are far slower — don't).  A few dummy
matmuls + one dummy activation at t~0 pre-warm the PE HAM clock gate
and the ACT table while the first loads land; GpSimd stays idle (its
TT is 4x slower and degrades concurrent DVE SBUF ops).
"""

import sys

if "/opt/trn_rl_repo" not in sys.path:
    sys.path.insert(0, "/opt/trn_rl_repo")

import numpy as np

GAMMA = 0.99
LAM = 0.95
C_COEF = GAMMA * LAM
K_COEF = (1.0 - LAM) / LAM  # e' = r + k*v ; ret = C - k*v
R_COEF = 1.0 / (1.0 - LAM) - 1.0  # adv = ret - R_COEF * (k*v)

B, S = 2048, 8192
N_CORES = 8
ROWS = B // N_CORES  # 256 rows per core
P = 128  # SBUF partitions = seq positions per block
NB = S // P  # 64 seq blocks per core
NCOL = NB * ROWS  # 16384 packed columns
PHW = ROWS  # phantom block width (256)
SG = 2048  # supergroup: 4 PSUM banks
N_SG = NCOL // SG  # 8 supergroups
WCLAMP = 1e-4  # zero fp16-subnormal-ish weights (horizon stays ~150+)

_CACHE: dict = {}


def _weights():
    idx = np.arange(P)
    d = idx[:, None] - idx[None, :]  # j - i  (j = contraction partition)
    w0 = np.where(d >= 0, C_COEF ** np.maximum(d, 0), 0.0)  # T0 (with diag)
    w0m = np.where(d > 0, C_COEF ** np.maximum(d, 0), 0.0)  # T0 - I
    w1 = C_COEF ** (128.0 + d)
    out = []
    for w in (w0, w0m, w1):
        w[np.abs(w) < WCLAMP] = 0.0
        out.append(w.astype(np.float16))
    return out


def _build():
    import concourse.bacc as bacc
    import concourse.mybir as mybir
    from concourse.tile import TileContext

    f16 = mybir.dt.float16
    f32 = mybir.dt.float32
    add = mybir.AluOpType.add
    mult = mybir.AluOpType.mult
    Copy = mybir.ActivationFunctionType.Copy

    nc = bacc.Bacc("TRN2", target_bir_lowering=False, name="gaeconv1")
    rp = nc.dram_tensor("rp", [P, NCOL + PHW], f16, kind="ExternalInput")
    mp = nc.dram_tensor("mp", [P, NCOL + PHW], f16, kind="ExternalInput")
    w0_d = nc.dram_tensor("w0", [P, P], f16, kind="ExternalInput")
    w0m_d = nc.dram_tensor("w0m", [P, P], f16, kind="ExternalInput")
    w1_d = nc.dram_tensor("w1", [P, P], f16, kind="ExternalInput")
    # per supergroup g: cols [4096g, 4096g+2048) = ret, [+2048, +4096) = adv
    o2d = nc.dram_tensor("o2", [P, 2 * NCOL], f16, kind="ExternalOutput")

    with TileContext(nc) as tc:
        with (
            tc.tile_pool(name="cpool", bufs=1) as cpool,
            tc.tile_pool(name="psum", bufs=2, space="PSUM") as psum,
            tc.tile_pool(name="opool", bufs=5) as opool,
        ):
            w0_t = cpool.tile([P, P], f16)
            w0m_t = cpool.tile([P, P], f16)
            w1_t = cpool.tile([P, P], f16)
            rp_t = cpool.tile([P, NCOL + PHW], f16)
            mp_t = cpool.tile([P, NCOL + PHW], f16)
            scratch = cpool.tile([P, P], f16)

            nc.sync.dma_start(out=w0_t[:, :], in_=w0_d[:, :])
            nc.sync.dma_start(out=w0m_t[:, :], in_=w0m_d[:, :])
            nc.sync.dma_start(out=w1_t[:, :], in_=w1_d[:, :])
            # loads split across both HWDGE rings (rp on sync, mp on
            # scalar); finer pieces up front so the PE's first supergroups
            # aren't starved during the DMA ramp; the tail chunk carries
            # the phantom block.
            bounds = [0, 1024, 2048, 3072, 4096, 6144, 8192, 10240, 12288,
                      14336, NCOL + PHW]
            for a, b in zip(bounds, bounds[1:]):
                nc.sync.dma_start(out=rp_t[:, a:b], in_=rp[:, a:b])
                nc.scalar.dma_start(out=mp_t[:, a:b], in_=mp[:, a:b])

            # ACT table pre-load + PE HAM warm-up while the loads land
            nc.scalar.activation(out=scratch[:, 0:1], in_=w0_t[:, 0:1], func=Copy)
            warm = psum.tile([P, SG // 2], f32, tag="psA")
            for _ in range(12):
                nc.tensor.matmul(
                    warm[:, 0:P], w0_t[:, :], w0_t[:, :], start=True, stop=True
                )

            H = SG // 2
            for g in range(1, N_SG + 1):
                X = (g - 1) * SG
                # two separate 2-bank PSUM tiles so the ScalarE and DVE
                # drains track as disjoint tensors and run in parallel
                # (same-tile access serializes under Tile's bank tracker)
                ps_lo = psum.tile([P, H], f32, tag="psA")
                ps_hi = psum.tile([P, H], f32, tag="psB")
                o2_t = opool.tile([P, 2 * SG], f16)
                for bk in range(4):
                    xb = X + 512 * bk
                    ph = ps_lo if bk < 2 else ps_hi
                    po = ph[:, 512 * (bk % 2) : 512 * (bk % 2) + 512]
                    nc.tensor.matmul(
                        po, w0_t[:, :], rp_t[:, xb : xb + 512],
                        start=True, stop=False,
                    )
                    nc.tensor.matmul(
                        po, w0m_t[:, :], mp_t[:, xb : xb + 512],
                        start=False, stop=False,
                    )
                    nc.tensor.matmul(
                        po, w1_t[:, :], rp_t[:, xb + PHW : xb + PHW + 512],
                        start=False, stop=False,
                    )
                    nc.tensor.matmul(
                        po, w1_t[:, :], mp_t[:, xb + PHW : xb + PHW + 512],
                        start=False, stop=True,
                    )
                # parallel drains: ACT on ps_lo while DVE works ps_hi, swap
                nc.scalar.activation(
                    out=o2_t[:, 0:H], in_=ps_lo[:, :], func=Copy
                )
                nc.vector.scalar_tensor_tensor(
                    out=o2_t[:, SG + H : 2 * SG],
                    in0=mp_t[:, X + H : X + SG],
                    scalar=-R_COEF,
                    in1=ps_hi[:, :],
                    op0=mult,
                    op1=add,
                )
                nc.scalar.activation(
                    out=o2_t[:, H:SG], in_=ps_hi[:, :], func=Copy
                )
                nc.vector.scalar_tensor_tensor(
                    out=o2_t[:, SG : SG + H],
                    in0=mp_t[:, X : X + H],
                    scalar=-R_COEF,
                    in1=ps_lo[:, :],
                    op0=mult,
                    op1=add,
                )
                # every sg's two output halves go to OPPOSITE rings, so each
                # ring carries exactly half the loads + half the stores and
                # flips direction once; drains finish ahead of the ring's
                # FIFO position, keeping both rings 100% busy to the end.
                nc.scalar.dma_start(
                    out=o2d[:, 2 * X : 2 * X + SG], in_=o2_t[:, 0:SG]
                )
                nc.sync.dma_start(
                    out=o2d[:, 2 * X + SG : 2 * X + 2 * SG],
                    in_=o2_t[:, SG : 2 * SG],
                )
    nc.finalize()
    return nc


def _get_nc():
    if "nc" not in _CACHE:
        _CACHE["nc"] = _build()
    return _CACHE["nc"]


def _pack(x):
    """[ROWS, S] -> [P, NB*ROWS] with seq in partitions per 128-block."""
    return np.ascontiguousarray(
        x.reshape(ROWS, NB, P).transpose(2, 1, 0).reshape(P, NCOL)
    )


def _unpack(xp):
    """[P, NB*ROWS] -> [ROWS, S]."""
    return xp.reshape(P, NB, ROWS).transpose(2, 1, 0).reshape(ROWS, S)


def _run(rewards, values, next_values, **spmd_kwargs):
    """Shard over cores, run the Bass kernel, return BassKernelResults."""
    from concourse.bass_utils import run_bass_kernel_spmd

    nc = _get_nc()
    r32 = np.asarray(rewards, dtype=np.float32)
    mk32 = np.asarray(values, dtype=np.float32) * np.float32(K_COEF)
    nv32 = np.asarray(next_values, dtype=np.float32)
    w0, w0m, w1 = _weights()
    zeros_ph = np.zeros((P, PHW), dtype=np.float16)
    in_maps = []
    for c in range(N_CORES):
        sl = slice(c * ROWS, (c + 1) * ROWS)
        ph = zeros_ph.copy()
        ph[0, :] = (nv32[sl] / np.float32(LAM)).astype(np.float16)
        rp = np.concatenate([_pack(r32[sl].astype(np.float16)), ph], 1)
        mpk = np.concatenate(
            [_pack(mk32[sl].astype(np.float16)), zeros_ph], 1
        )
        in_maps.append(
            {
                "rp": np.ascontiguousarray(rp),
                "mp": np.ascontiguousarray(mpk),
                "w0": w0,
                "w0m": w0m,
                "w1": w1,
            }
        )
    return run_bass_kernel_spmd(
        nc, in_maps, core_ids=list(range(N_CORES)), **spmd_kwargs
    )


def _gather(res):
    """Unshard: de-interleave per-supergroup ret/adv, unpack, upcast."""
    returns = np.empty((B, S), dtype=np.float32)
    advantages = np.empty((B, S), dtype=np.float32)
    for c in range(N_CORES):
        sl = slice(c * ROWS, (c + 1) * ROWS)
        o2 = res.results[c]["o2"].reshape(P, N_SG, 2, SG)
        retp = np.ascontiguousarray(o2[:, :, 0, :]).reshape(P, NCOL)
        advp = np.ascontiguousarray(o2[:, :, 1, :]).reshape(P, NCOL)
        returns[sl] = _unpack(retp.astype(np.float32))
        advantages[sl] = _unpack(advp.astype(np.float32))
    return advantages, returns


def kernel(rewards, values, next_values):
    res = _run(rewards, values, next_values)
    return _gather(res)
